# revision 10
# baseline (speedup 1.0000x reference)
"""DeepFM forward on 8 Trainium2 NeuronCores.

Data-parallel: batch 8192 -> 1024 samples/core; tables replicated.

Math restructuring (weight-only preprocessing on host):
  logit_b = fm_b + wide_b + deep_b + b_ffn
  fm_b   = sum_{i<j} w2_ij <e_i,e_j> = sum_k lam_k sum_f (V^T E_b)^2[k,f]
           (A = sym(w2)/2 = V diag(lam) V^T, exact float64 eigh)
  wide_b = sum_{t,f} W3[t,f] e_{b,t,f}
  deep_b = sum_{t,k} W1[t,k] relu(emb @ w_deep + b_deep)[x_bt, k]
  out_b  = sigmoid(logit_b)

Device dataflow per core (chunks of 64 samples):
  1. One per-sample indirect DMA (gpsimd, offsets [128,1] int32 = x[b,:])
     gathers the 128B combined row [emb*32 fp8 x64 | relu-feat bf16 x32]
     per token into sel[t-partition, b, 128] (partition-indirect mode is
     the only indirect-DMA shape that is correct on this HW).
  2. TensorE: P = V^T E (V stationary bf16, E fp8 view) -> PSUM.
  3. ScalarE square -> bf16, DVE grouped reduce -> sq_acc.
  4. DVE mult+reduce vs fixed W3/W1 patterns -> wide/deep accs
     (W3 pre-scaled x32 to stay in fp8 normal range; undone via the
     1/EMB_SCALE^2 lhsT in the final matmul).
  5. Per-chunk lam/ones matmuls fold accs into the logit PSUM (no serial
     tail); final sigmoid(+b_ffn), DMA out. Host concatenates cores.

Perf note: runtime is gather-mechanism-bound. Each per-sample indirect DMA
costs ~1.4us on the Pool Q7 (994ns fixed ucode + ~310ns launch gap), x1024
samples/core = ~1.43ms floor. dma_gather (InstDMAGatherAnt) would amortize
the fixed cost but its int16 indices cannot address VOCAB=100000 rows
(max 32767), and range-splitting costs 4x traffic or 4x combine compute.
"""

import os
import numpy as np

import concourse.bass as bass
import concourse.mybir as mybir
from concourse import bacc
from concourse.tile import TileContext
from concourse.bass_utils import run_bass_kernel_spmd

BS, TS, VOCAB, F = 8192, 100, 100000, 64
K = 32
NCORES = 8
SPC = BS // NCORES
CN = 64                  # samples per chunk
GRP = 8                  # samples per matmul
EMB_SCALE = 32.0
NPACK = VOCAB // 4       # 25000 quad packs
ROWB = 128               # bytes per packed row
PACKB = 4 * ROWB         # 512

U8 = mybir.dt.uint8
FP8 = mybir.dt.float8e4
BF16 = mybir.dt.bfloat16
F32 = mybir.dt.float32
I16 = mybir.dt.int16

_cached = {}


def build_nc(spc=SPC, cn=CN):
    n_chunks = spc // cn
    ni = cn * 128            # gather list entries per chunk (padded tokens)
    nc = bacc.Bacc("TRN2", target_bir_lowering=False, debug=False,
                   num_devices=NCORES)
    rtab = nc.dram_tensor("rtab", [VOCAB, ROWB], U8, kind="ExternalInput")
    xt32 = nc.dram_tensor("xt32", [128, spc], mybir.dt.int32,
                          kind="ExternalInput")
    vmat = nc.dram_tensor("vmat", [128, TS], BF16, kind="ExternalInput")
    lam = nc.dram_tensor("lam", [TS, 1], F32, kind="ExternalInput")
    onesv = nc.dram_tensor("onesv", [128, 1], F32, kind="ExternalInput")
    onesw = nc.dram_tensor("onesw", [128, 1], F32, kind="ExternalInput")
    w3t = nc.dram_tensor("w3t", [128, F], FP8, kind="ExternalInput")
    w1t = nc.dram_tensor("w1t", [128, K], BF16, kind="ExternalInput")
    bffn = nc.dram_tensor("bffn", [1, 1], F32, kind="ExternalInput")
    y = nc.dram_tensor("y", [1, spc], F32, kind="ExternalOutput")

    with TileContext(nc) as tc:
        with (
            tc.tile_pool(name="const", bufs=1) as cpool,
            tc.tile_pool(name="acc", bufs=1) as apool,
            tc.tile_pool(name="pk", bufs=2) as kpool,
            tc.tile_pool(name="sel", bufs=3) as spool,
            tc.tile_pool(name="sq", bufs=3) as qpool,
            tc.tile_pool(name="prod", bufs=2) as rpool,
            tc.tile_pool(name="psum", bufs=2, space="PSUM") as ppool,
            tc.tile_pool(name="psuml", bufs=1, space="PSUM") as lpool,
        ):
            xt_sb = cpool.tile([128, spc], mybir.dt.int32)
            # chunk 0's offsets land first so gathers start immediately
            nc.sync.dma_start(out=xt_sb[:, 0:cn], in_=xt32.ap()[:, 0:cn])
            nc.sync.dma_start(out=xt_sb[:, cn:], in_=xt32.ap()[:, cn:])
            v_sb = cpool.tile([128, TS], BF16)
            nc.sync.dma_start(out=v_sb[:], in_=vmat.ap())
            lam_sb = cpool.tile([TS, 1], F32)
            nc.sync.dma_start(out=lam_sb[:], in_=lam.ap())
            ones_sb = cpool.tile([128, 1], F32)
            nc.sync.dma_start(out=ones_sb[:], in_=onesv.ap())
            onesw_sb = cpool.tile([128, 1], F32)
            nc.sync.dma_start(out=onesw_sb[:], in_=onesw.ap())
            w3_sb = cpool.tile([128, F], FP8)
            nc.sync.dma_start(out=w3_sb[:], in_=w3t.ap())
            w1_sb = cpool.tile([128, K], BF16)
            nc.sync.dma_start(out=w1_sb[:], in_=w1t.ap())
            bffn_sb = cpool.tile([1, 1], F32)
            nc.sync.dma_start(out=bffn_sb[:], in_=bffn.ap())

            sq_acc = apool.tile([TS, spc], F32)
            w_acc = apool.tile([128, spc], F32)
            d_acc = apool.tile([128, spc], F32)
            pl = lpool.tile([1, spc], F32, space="PSUM")

            for c in range(n_chunks):
                c0 = c * cn
                sel = spool.tile([128, cn, ROWB], U8, tag="sel")
                for b in range(cn):
                    nc.gpsimd.indirect_dma_start(
                        out=sel[:, b, :], out_offset=None,
                        in_=rtab.ap(),
                        in_offset=bass.IndirectOffsetOnAxis(
                            ap=xt_sb[:, c0 + b:c0 + b + 1], axis=0))

                sel_e = sel[:, :, 0:F].bitcast(FP8)
                sel_h = sel[:, :, F:F + 2 * K].bitcast(BF16)

                for g in range(cn // GRP):
                    p = ppool.tile([TS, GRP * F], F32, space="PSUM")
                    nc.tensor.matmul(
                        out=p[:], lhsT=v_sb[:],
                        rhs=sel_e[:, g * GRP:(g + 1) * GRP, :],
                        start=True, stop=True)
                    sq = qpool.tile([TS, GRP, F], BF16)
                    nc.scalar.activation(
                        sq[:], p[:].rearrange("p (g f) -> p g f", f=F),
                        mybir.ActivationFunctionType.Square)
                    nc.vector.tensor_reduce(
                        out=sq_acc[:, c0 + g * GRP:c0 + (g + 1) * GRP],
                        in_=sq[:], axis=mybir.AxisListType.X,
                        op=mybir.AluOpType.add)

                # wide/deep DVE per half-chunk so the work overlaps this
                # chunk's own gathers instead of serializing after them
                wprod = rpool.tile([128, cn, F], BF16, tag="wp")
                dprod = rpool.tile([128, cn, K], BF16, tag="dp")
                hn = cn // 4
                for h0 in range(0, cn, hn):
                    hs = slice(h0, h0 + hn)
                    # wide: e (fp8) * W3 pattern, reduce over f
                    nc.vector.tensor_tensor(
                        out=wprod[:, hs, :], in0=sel_e[:, hs, :],
                        in1=w3_sb[:].rearrange("p (a f) -> p a f", a=1).to_broadcast([128, hn, F]),
                        op=mybir.AluOpType.mult)
                    nc.vector.tensor_reduce(
                        out=w_acc[:, c0 + h0:c0 + h0 + hn], in_=wprod[:, hs, :],
                        axis=mybir.AxisListType.X, op=mybir.AluOpType.add)
                    # deep: h (bf16) * W1 pattern, reduce over k
                    nc.vector.tensor_tensor(
                        out=dprod[:, hs, :], in0=sel_h[:, hs, :],
                        in1=w1_sb[:].rearrange("p (a k) -> p a k", a=1).to_broadcast([128, hn, K]),
                        op=mybir.AluOpType.mult)
                    nc.vector.tensor_reduce(
                        out=d_acc[:, c0 + h0:c0 + h0 + hn], in_=dprod[:, hs, :],
                        axis=mybir.AxisListType.X, op=mybir.AluOpType.add)

                # fold this chunk's accs into the logit PSUM now, so the
                # final reduction isn't a serial tail after the last gather
                sl = slice(c0, c0 + cn)
                nc.tensor.matmul(out=pl[:, sl], lhsT=lam_sb[:],
                                 rhs=sq_acc[:, sl], start=True, stop=False)
                nc.tensor.matmul(out=pl[:, sl], lhsT=onesw_sb[:],
                                 rhs=w_acc[:, sl], start=False, stop=False)
                nc.tensor.matmul(out=pl[:, sl], lhsT=ones_sb[:],
                                 rhs=d_acc[:, sl], start=False, stop=True)

            y_sb = cpool.tile([1, spc], F32)
            nc.scalar.activation(y_sb[:], pl[:],
                                 mybir.ActivationFunctionType.Sigmoid,
                                 bias=bffn_sb[:, :])
            nc.sync.dma_start(out=y.ap(), in_=y_sb[:])

    nc.compile()
    return nc


def _host_prep(x, emb, w_deep, b_deep, w_ffn, b_ffn, spc=SPC):
    x = np.asarray(x)
    emb = np.asarray(emb, dtype=np.float32)
    w_deep = np.asarray(w_deep, dtype=np.float32)
    b_deep = np.asarray(b_deep, dtype=np.float32)
    w_ffn = np.asarray(w_ffn, dtype=np.float32).reshape(-1)
    b_ffn = np.asarray(b_ffn, dtype=np.float32).reshape(-1)

    n_deep = TS * K
    n_fm = TS * (TS - 1) // 2
    w1 = w_ffn[:n_deep].reshape(TS, K)
    w2 = w_ffn[n_deep:n_deep + n_fm].astype(np.float64)
    w3 = w_ffn[n_deep + n_fm:].reshape(TS, F)

    iu, ju = np.triu_indices(TS, k=1)
    A = np.zeros((TS, TS), dtype=np.float64)
    A[iu, ju] = w2 / 2
    A = A + A.T
    lam, V = np.linalg.eigh(A)

    fp8_np = mybir.dt.np(FP8)
    bf16_np = mybir.dt.np(BF16)

    emb8 = (emb * EMB_SCALE).astype(fp8_np)                       # [V, 64]
    hfeat = np.maximum(emb.astype(np.float64) @ w_deep + b_deep,
                       0.0).astype(bf16_np)                        # [V, 32]
    rows = np.zeros((VOCAB, ROWB), dtype=np.uint8)
    rows[:, 0:F] = emb8.view(np.uint8)
    rows[:, F:F + 2 * K] = hfeat.view(np.uint8).reshape(VOCAB, 2 * K)
    rtab = np.ascontiguousarray(rows)

    vz = np.zeros((128, TS), dtype=bf16_np)
    vz[:TS, :] = V.astype(bf16_np)
    lam_dev = (lam / (EMB_SCALE * EMB_SCALE)).astype(np.float32).reshape(TS, 1)
    onesz = np.zeros((128, 1), dtype=np.float32)
    onesz[:TS] = 1.0
    w3z = np.zeros((128, F), dtype=fp8_np)
    w3z[:TS] = (w3 * EMB_SCALE).astype(fp8_np)
    w1z = np.zeros((128, K), dtype=bf16_np)
    w1z[:TS] = w1.astype(bf16_np)

    shared = {
        "rtab": rtab,
        "vmat": vz, "lam": lam_dev, "onesv": onesz,
        "onesw": onesz / (EMB_SCALE * EMB_SCALE),
        "w3t": w3z, "w1t": w1z,
        "bffn": b_ffn.reshape(1, 1).astype(np.float32),
    }
    xi = x.astype(np.int64)
    ncores = xi.shape[0] // spc
    in_maps = []
    for c in range(ncores):
        xs = xi[c * spc:(c + 1) * spc]                    # [spc, TS]
        xpad = np.zeros((spc, 128), dtype=np.int32)       # padded tokens -> 0
        xpad[:, :TS] = xs
        xt32 = np.ascontiguousarray(xpad.T)               # [128(t), spc]
        in_maps.append({"xt32": xt32, **shared})
    return in_maps


def kernel(x, emb, w_deep, b_deep, w_ffn, b_ffn):
    if "nc" not in _cached:
        _cached["nc"] = build_nc()
    nc = _cached["nc"]
    in_maps = _host_prep(x, emb, w_deep, b_deep, w_ffn, b_ffn)
    trace = os.environ.get("KERNEL_TRACE", "") == "1"
    res = run_bass_kernel_spmd(nc, in_maps, core_ids=list(range(NCORES)),
                               trace=trace)
    if trace and res.exec_time_ns is not None:
        print(f"HW exec time: {res.exec_time_ns} ns")
        print(f"mean exec time: {res.mean_exec_time_ns} ns")
        if res.instructions_and_trace:
            print(f"trace: {res.instructions_and_trace[1]}")
    out = np.concatenate([res.results[c]["y"].reshape(SPC)
                          for c in range(NCORES)])
    return out.reshape(BS, 1).astype(np.float32)

